# revision 10
# baseline (speedup 1.0000x reference)
"""CMoE hash-routed expert FFN on 8 NeuronCores (expert-parallel).

Host side: hash routing e = (token_id % 5099) % 64, first-come slot
assignment with capacity 512, scatter into a per-expert [E, C, D] buffer,
then PREPACK every operand into its exact SBUF layout (partition dim
first, rows fully contiguous in DRAM) so each DMA is one big 2D transfer
with 4-14KB per-partition rows that the DGE coalesces into large packets.
8 experts per core.  Device side, per expert:
    h  = relu(A @ Wk^T)^2        [C, F]   bf16 matmuls
    kv = h @ Wv^T                [C, D]   bf16 matmuls
    r  = sigmoid(A @ Wr^T)       [C, D]   fp8 e4m3 DoubleRow matmuls (2x)
    out = r * kv                          stored bf16
computed in transposed form (contraction dim on SBUF partitions), fp32
PSUM accumulation.  The r path quantizes A and 1024*Wr to fp8 e4m3
(sigmoid compresses the quantization error ~5x; measured end-to-end
rel-err ~0.009 vs the 0.02 gate) and runs double-pumped DoubleRow
matmuls, saving ~12.5% of PE streaming cycles.  Wk is split into two
f-halves DMA'd on two different rings in parallel so the first h matmul
group is gated on half the bytes.  Host gathers each token's slot back
out of the [E, D, C]-equivalent output and zeroes dropped tokens.
"""

import numpy as np
import ml_dtypes

import concourse.bass as bass
import concourse.mybir as mybir
import concourse.tile as tile
from concourse import bacc
from concourse.bass import ts
from concourse.bass_utils import run_bass_kernel_spmd

HASH_PRIME = 5099
B, T, D, F, E = 8, 4096, 512, 1792, 64
S = B * T
C = 512  # capacity = max(4, ceil(S/E))
N_CORES = 8
E_LOC = E // N_CORES  # experts per core
KD = D // 128   # 4  contraction tiles over D
KF = F // 128   # 14 contraction tiles over F
FH = F // 2     # 896 f-half width

BF16 = mybir.dt.bfloat16
F8 = mybir.dt.float8e4
F32 = mybir.dt.float32
R_SCALE = 1024.0  # Wr prescale so fp8 e4m3 stays in normal range

_NC = None  # cached compiled Bass program
LAST_RESULT = None  # BassKernelResults of the most recent run (for test.py)


def _build_nc(e_loc=E_LOC, d=D, f=F, c=C):
    """One SPMD program: each core computes e_loc experts' FFN."""
    nc = bacc.Bacc("TRN2", target_bir_lowering=False, debug=False,
                   num_devices=N_CORES)

    a_bf = nc.dram_tensor("a_bf", [e_loc, 128, KD, c], BF16, kind="ExternalInput")
    a_f8 = nc.dram_tensor("a_f8", [e_loc, 128, KD, c], F8, kind="ExternalInput")
    # wk in three f-chunks (896|512|384 cols) so h's later ft-groups gate
    # on separately-DMA'd pieces that can ride different rings
    wk0_t = nc.dram_tensor("wk0", [e_loc, 128, KD, FH], BF16, kind="ExternalInput")
    wk1a_t = nc.dram_tensor("wk1a", [e_loc, 128, KD, 512], BF16, kind="ExternalInput")
    wk1b_t = nc.dram_tensor("wk1b", [e_loc, 128, KD, 384], BF16, kind="ExternalInput")
    wr = nc.dram_tensor("wr", [e_loc, 128, KD, d], F8, kind="ExternalInput")
    wv = nc.dram_tensor("wv", [e_loc, 128, KF, d], BF16, kind="ExternalInput")
    out = nc.dram_tensor("out", [e_loc, 128, KD, c], BF16, kind="ExternalOutput")

    with tile.TileContext(nc) as tc:
        with (
            tc.tile_pool(name="wts", bufs=2) as wts,
            tc.tile_pool(name="acts", bufs=2) as acts,
            tc.tile_pool(name="ph", bufs=3, space="PSUM") as ph,
            tc.tile_pool(name="pr", bufs=3, space="PSUM") as pr,
            tc.tile_pool(name="pkv", bufs=2, space="PSUM") as pkv,
        ):
            tiles = {}
            wvs = {}

            # Warm the PE (HAM ramps the clock over the first ~10us of
            # sustained activity) with matmuls on scratch data while the
            # first input DMAs stream; the result is never read.
            warm_l = wts.tile([128, 128], BF16, tag="warm_l")
            warm_r = wts.tile([128, c], BF16, tag="warm_r")
            nc.any.memset(warm_l[:], 0.0)
            nc.any.memset(warm_r[:], 0.0)
            for _ in range(16):
                warm_p = pr.tile([128, c], F32, tag="psr")
                nc.tensor.matmul(warm_p[:], lhsT=warm_l[:], rhs=warm_r[:],
                                 start=True, stop=True)

            def loads(e):
                # h(e) gates on a_bf + the two wk halves, streamed on
                # different rings in parallel.  Expert 0 is special-cased
                # for startup latency: sync's HWDGE delivers its first
                # byte ~1.5us after issue vs ~3/4.5us for scalar/gpsimd,
                # so the critical wk halves ride sync; the small fp8 r
                # operands go first on gpsimd so the scheduler's hoisted
                # r(0) group never blocks the queue.
                t_at = wts.tile([128, KD, c], BF16, tag="at")
                t_wk0 = wts.tile([128, KD, FH], BF16, tag="wk0")
                t_wk1a = wts.tile([128, KD, 512], BF16, tag="wk1a")
                t_wk1b = wts.tile([128, KD, 384], BF16, tag="wk1b")
                t_a8 = wts.tile([128, KD, c], F8, tag="a8")
                t_wr = wts.tile([128, KD, d], F8, tag="wr")
                if e == 0:
                    # gpsimd (SWDGE) crawls at startup, so it only gets
                    # small/late-needed pieces; wr8 rides scalar behind
                    # a_bf so the scheduler-hoisted r(0) group never
                    # blocks the PE queue waiting on it.
                    nc.gpsimd.dma_start(t_a8[:], a_f8[e])
                    nc.sync.dma_start(t_wk0[:], wk0_t[e])
                    nc.sync.dma_start(t_wk1a[:], wk1a_t[e])
                    nc.gpsimd.dma_start(t_wk1b[:], wk1b_t[e])
                    nc.scalar.dma_start(t_at[:], a_bf[e])
                    nc.scalar.dma_start(t_wr[:], wr[e])
                else:
                    nc.sync.dma_start(t_at[:], a_bf[e])
                    nc.scalar.dma_start(t_wk0[:], wk0_t[e])
                    nc.gpsimd.dma_start(t_wk1a[:], wk1a_t[e])
                    nc.gpsimd.dma_start(t_wk1b[:], wk1b_t[e])
                    nc.scalar.dma_start(t_a8[:], a_f8[e])
                    nc.gpsimd.dma_start(t_wr[:], wr[e])
                tiles[e] = (t_at, t_wk0, t_wk1a, t_wk1b, t_a8, t_wr)

            def load_wv(e):
                # wv(e) rides sync behind the expert-0 wk chunks /
                # behind a_bf(e) for later experts, landing an expert
                # cycle before kv(e) reads it.
                t_wv = wts.tile([128, KF, d], BF16, tag="wv")
                nc.sync.dma_start(t_wv[:], wv[e])
                wvs[e] = t_wv

            def emit(e):
                t_at, t_wk0, t_wk1a, t_wk1b, t_a8, t_wr = tiles.pop(e)

                # h^T[f, c] = (relu(Wk^T.T @ A^T))^2, bf16 for matmul 2
                hb = acts.tile([128, KF, c], BF16, tag="hb")
                for ft in range(KF):
                    if ft < 7:
                        wkt, fo = t_wk0, ft
                    elif ft < 11:
                        wkt, fo = t_wk1a, ft - 7
                    else:
                        wkt, fo = t_wk1b, ft - 11
                    psum_h = ph.tile([128, c], F32, tag="psh")
                    for kt in range(KD):
                        nc.tensor.matmul(
                            psum_h[:],
                            lhsT=wkt[:, kt, ts(fo, 128)],
                            rhs=t_at[:, kt, :],
                            start=(kt == 0),
                            stop=(kt == KD - 1),
                        )
                    nc.scalar.activation(hb[:, ft, :], psum_h[:],
                                         mybir.ActivationFunctionType.Relu)
                    nc.vector.tensor_mul(hb[:, ft, :], hb[:, ft, :], hb[:, ft, :])

                # r^T[g, c] = sigmoid((1024*Wr)^T.T @ A^T / 1024), fp8
                # DoubleRow: each matmul contracts a pair of k-tiles.
                sig = acts.tile([128, KD, c], F32, tag="sig")
                for gt in range(KD):
                    psum_r = pr.tile([128, c], F32, tag="psr")
                    for kp in range(KD // 2):
                        nc.tensor.matmul(
                            psum_r[:],
                            lhsT=t_wr[:, 2 * kp:2 * kp + 2, ts(gt, 128)],
                            rhs=t_a8[:, 2 * kp:2 * kp + 2, :],
                            start=(kp == 0),
                            stop=(kp == KD // 2 - 1),
                            perf_mode=mybir.MatmulPerfMode.DoubleRow,
                        )
                    nc.scalar.activation(sig[:, gt, :], psum_r[:],
                                         mybir.ActivationFunctionType.Sigmoid,
                                         scale=1.0 / R_SCALE)

                # kv^T[dd, c] = Wv^T.T @ h^T ; out = sig * kv, stored bf16
                ob = acts.tile([128, KD, c], BF16, tag="ob")
                for dt in range(KD):
                    psum_kv = pkv.tile([128, c], F32, tag="pskv")
                    for ft in range(KF):
                        nc.tensor.matmul(
                            psum_kv[:],
                            lhsT=wvs[e][:, ft, ts(dt, 128)],
                            rhs=hb[:, ft, :],
                            start=(ft == 0),
                            stop=(ft == KF - 1),
                        )
                    nc.vector.tensor_mul(ob[:, dt, :], psum_kv[:], sig[:, dt, :])
                    # the last expert's stores ride the by-then-idle sync
                    # ring to shorten the kernel tail
                    dst = out[e][:, dt, :]
                    if e == e_loc - 1:
                        nc.sync.dma_start(dst, ob[:, dt, :])
                    else:
                        nc.gpsimd.dma_start(dst, ob[:, dt, :])
                wvs.pop(e)

            loads(0)
            load_wv(0)
            if e_loc > 1:
                loads(1)
            for e in range(e_loc):
                emit(e)
                if e + 1 < e_loc:
                    load_wv(e + 1)
                if e + 2 < e_loc:
                    loads(e + 2)

    nc.compile()
    return nc


def _route(token_ids):
    tid = token_ids.reshape(S).astype(np.int64)
    e_idx = (tid % HASH_PRIME) % E
    order = np.argsort(e_idx, kind="stable")
    sorted_e = e_idx[order]
    starts = np.searchsorted(sorted_e, np.arange(E))
    pos = np.empty(S, np.int64)
    pos[order] = np.arange(S) - starts[sorted_e]
    kept = pos < C
    return e_idx, pos, kept


def kernel(x, token_ids, Wk, Wr, Wv):
    global _NC, LAST_RESULT
    if _NC is None:
        _NC = _build_nc()

    e_idx, pos, kept = _route(token_ids)

    bf16 = ml_dtypes.bfloat16
    f8 = ml_dtypes.float8_e4m3
    xf = np.ascontiguousarray(x, dtype=np.float32).reshape(S, D)
    disp = np.zeros((E, C, D), np.float32)
    disp[e_idx[kept], pos[kept]] = xf[kept]

    # SBUF layouts: partition dim first, per-partition rows contiguous.
    at4 = np.ascontiguousarray(
        disp.reshape(E, C, KD, 128).transpose(0, 3, 2, 1))      # [E,128,KD,C]
    a_bf = at4.astype(bf16)
    a_f8 = at4.astype(f8)
    wkT = np.asarray(Wk, np.float32).reshape(E, F, KD, 128) \
        .transpose(0, 3, 2, 1)                                  # [E,128,KD,F]
    wk0p = np.ascontiguousarray(wkT[..., 0:FH]).astype(bf16)
    wk1a = np.ascontiguousarray(wkT[..., FH:FH + 512]).astype(bf16)
    wk1b = np.ascontiguousarray(wkT[..., FH + 512:]).astype(bf16)
    wr8 = (np.ascontiguousarray(
        np.asarray(Wr, np.float32).reshape(E, D, KD, 128)
        .transpose(0, 3, 2, 1)) * R_SCALE).astype(f8)           # [E,128,KD,D]
    wvp = np.ascontiguousarray(
        np.asarray(Wv, np.float32).reshape(E, D, KF, 128)
        .transpose(0, 3, 2, 1)).astype(bf16)                    # [E,128,KF,D]

    in_maps = [
        {
            "a_bf": a_bf[i * E_LOC:(i + 1) * E_LOC],
            "a_f8": a_f8[i * E_LOC:(i + 1) * E_LOC],
            "wk0": wk0p[i * E_LOC:(i + 1) * E_LOC],
            "wk1a": wk1a[i * E_LOC:(i + 1) * E_LOC],
            "wk1b": wk1b[i * E_LOC:(i + 1) * E_LOC],
            "wr": wr8[i * E_LOC:(i + 1) * E_LOC],
            "wv": wvp[i * E_LOC:(i + 1) * E_LOC],
        }
        for i in range(N_CORES)
    ]

    LAST_RESULT = run_bass_kernel_spmd(_NC, in_maps, list(range(N_CORES)))
    outp = np.concatenate(
        [np.asarray(LAST_RESULT.results[i]["out"]) for i in range(N_CORES)],
        axis=0)                                                 # [E,128,KD,C]
    out_full = np.ascontiguousarray(
        outp.transpose(0, 2, 1, 3)).reshape(E, D, C).astype(np.float32)

    yf = out_full[e_idx, :, np.minimum(pos, C - 1)]
    yf[~kept] = 0.0
    return np.ascontiguousarray(yf.reshape(B, T, D), dtype=np.float32)
